# revision 19
# baseline (speedup 1.0000x reference)
"""MXFP4-quantized linear kernel for Trainium2 (8 NeuronCores, SPMD).

Problem: out = quant_mxfp4(x) @ W.T + bias
  x [2, 4096, 4096] f32, W [11008, 4096] f32, bias [11008] f32 -> out [2, 4096, 11008] f32

Strategy (data-parallel over rows of x; the sharding hint allows this):
  - Host: flatten x to [8192, 4096], shard rows 8 ways; pre-transpose W to
    WT [4096, 11008] and cast fp16 (static weight preprocessing).
  - Each core: quantize its x shard (dynamic per-32-block MXFP4) on-chip,
    work spread across DVE/ACT/GPSIMD; transpose quantized fp16 tiles to
    K-major via PE identity-matmul transposes (PSUM bounce, SWDGE copy-out);
    dense fp16 GEMM (fp32 PSUM accumulate) against streamed WT tiles; bias
    is injected by a K=1 ones x bias matmul that opens each accumulation
    group, so PSUM drains are pure copies. No collectives.

MXFP4 snap (branch-free, in 2x space so the grid is {0,1,2,3,4,6,8,12}):
  scale_h = fp16(amax/6)/2 exactly; w = x * (1/scale_h)
  high |w|>=2: Veltkamp split with 2^22+1 -> RNE to 2-bit significand
  low  |w|< 2: (w + 1.5*2^23) - 1.5*2^23 -> RNE to integer
  blend via copy_predicated on mask = |w| < sqrt(8); x_q = s * scale_h (f16)
Ties (exact fp midpoints) round to-even vs reference to-lower: measure-zero.
"""
import sys

try:
    import concourse  # noqa: F401
except ImportError:
    sys.path.insert(0, "/opt/trn_rl_repo")

import numpy as np

import concourse.bacc as bacc
import concourse.mybir as mybir
from concourse import tile
from concourse.masks import make_identity
from concourse.bass_utils import run_bass_kernel_spmd

F32, F16 = mybir.dt.float32, mybir.dt.float16
U8 = mybir.dt.uint8
ACT = mybir.ActivationFunctionType
ALU = mybir.AluOpType

CV = float(2**22 + 1)      # Veltkamp split constant -> 2-bit significand RNE
CR = float(1.5 * 2**22)    # RNE-to-multiple-of-0.5 trick constant
THR = 1.4142135            # low/high switch point, anywhere in (1, 1.75) works

N_CORES = 8
B, S, K, N = 2, 4096, 4096, 11008
M = B * S                  # 8192
MS = M // N_CORES          # 1024 rows per core
QC = 512                   # quant chunk width (along K)


def build_program(Ms=MS, Kd=K, Nd=N, wt_bufs=64, early_nc=2):
    """Build the SPMD Bass program for one core (same program on all cores)."""
    nc = bacc.Bacc("TRN2", target_bir_lowering=False, debug=False)
    x = nc.dram_tensor("x", [Ms, Kd], F32, kind="ExternalInput")
    wt = nc.dram_tensor("wt", [Kd, Nd], F16, kind="ExternalInput")
    bias = nc.dram_tensor("bias", [Nd], F32, kind="ExternalInput")
    out = nc.dram_tensor("out", [Ms, Nd], F32, kind="ExternalOutput")

    MT = Ms // 128          # m-tiles per core
    KT = Kd // 128          # k-tiles
    NB = QC // 32           # quant blocks per chunk
    QCH = Kd // QC          # quant chunks per m-tile
    TPC = QC // 128         # transposes per chunk

    nchunks = []
    n0 = 0
    while n0 < Nd:
        nw = min(512, Nd - n0)
        nchunks.append((n0, nw))
        n0 += nw
    early_nc = min(early_nc, len(nchunks))

    with tile.TileContext(nc) as tc:
        with (
            tc.tile_pool(name="xqt", bufs=1) as xqt_pool,
            tc.tile_pool(name="xin", bufs=4) as xin_pool,
            tc.tile_pool(name="qtmp", bufs=16) as qtmp_pool,
            tc.tile_pool(name="qsmall", bufs=6) as qsmall_pool,
            tc.tile_pool(name="xqc", bufs=6) as xqc_pool,
            tc.tile_pool(name="wtp", bufs=wt_bufs) as wt_pool,
            tc.tile_pool(name="outp", bufs=2) as out_pool,
            tc.tile_pool(name="bnc", bufs=2) as bias_pool,
            tc.tile_pool(name="cst", bufs=1) as const_pool,
            tc.tile_pool(name="psum", bufs=5, space="PSUM") as psum_pool,
            tc.tile_pool(name="psumt", bufs=3, space="PSUM") as psumt_pool,
        ):
            ident = const_pool.tile([128, 128], F16, tag="ident")
            make_identity(nc, ident[:])

            # persistent K-major quantized activations as ONE tensor, m-tile
            # major: [128, MT*Kd] f16; (mt, k) tile at cols mt*Kd + k*128
            xqT = xqt_pool.tile([128, MT * Kd], F16, tag="xqT")

            def lhsT(k, mt):
                return xqT[:, mt * Kd + k * 128: mt * Kd + (k + 1) * 128]

            # ---- Phase A: quantize x, m-tile by m-tile ----
            # per chunk: DVE{reduce, r2, s, pred, mask, final}
            #            ACT{sc16, sch, c, u, sL, abs}  GPS{w, d, T-copy-dma}
            #            PE{4 transposes}
            KB = Kd // 32       # amax blocks per m-tile
            for mt in range(MT):
                # batched per-m-tile scale computation: 8 chunk reduces into
                # one amax tile, then ONE sc16 + ONE reciprocal
                xins = []
                amax_mt = qsmall_pool.tile([128, KB], F32, tag="amax", bufs=2,
                                           name=f"amax{mt}")
                sc16_mt = qsmall_pool.tile([128, KB], F16, tag="sc16", bufs=2,
                                           name=f"sc16{mt}")
                r2_mt = qsmall_pool.tile([128, KB], F32, tag="r2", bufs=2,
                                         name=f"r2{mt}")
                for q in range(QCH):
                    k0 = q * QC
                    xin = xin_pool.tile([128, QC], F32, tag="xin", bufs=8, name=f"xin{mt}_{q}")
                    nc.sync.dma_start(out=xin[:], in_=x[mt * 128:(mt + 1) * 128, k0:k0 + QC])
                    nc.vector.tensor_reduce(
                        out=amax_mt[:, q * NB:(q + 1) * NB],
                        in_=xin.rearrange("p (b c) -> p b c", c=32),
                        axis=mybir.AxisListType.X, op=ALU.max,
                        apply_absolute_value=True)
                    xins.append(xin)
                    if mt == 0:
                        # per-chunk smalls: chunk q flows without waiting for
                        # the whole m-tile's reduces (cuts kernel startup)
                        nc.scalar.activation(
                            out=sc16_mt[:, q * NB:(q + 1) * NB],
                            in_=amax_mt[:, q * NB:(q + 1) * NB],
                            func=ACT.Copy, scale=float(1.0 / 6.0))
                        nc.vector.reciprocal(
                            out=r2_mt[:, q * NB:(q + 1) * NB],
                            in_=sc16_mt[:, q * NB:(q + 1) * NB])
                if mt > 0:
                    nc.scalar.activation(out=sc16_mt[:], in_=amax_mt[:], func=ACT.Copy,
                                         scale=float(1.0 / 6.0))
                    nc.vector.reciprocal(out=r2_mt[:], in_=sc16_mt[:])

                for q in range(QCH):
                    xin = xins[q]
                    r2 = r2_mt[:, q * NB:(q + 1) * NB]
                    sc16 = sc16_mt[:, q * NB:(q + 1) * NB]

                    w = qtmp_pool.tile([128, QC], F32, tag="qt32", bufs=16, name=f"w{mt}_{q}")
                    nc.gpsimd.tensor_tensor(
                        out=w.rearrange("p (b c) -> p b c", c=32),
                        in0=xin.rearrange("p (b c) -> p b c", c=32),
                        in1=r2.unsqueeze(2).broadcast_to([128, NB, 32]),
                        op=ALU.mult)

                    c = qtmp_pool.tile([128, QC], F32, tag="qt32", bufs=16, name=f"c{mt}_{q}")
                    nc.scalar.activation(out=c[:], in_=w[:], func=ACT.Copy, scale=CV)
                    d = qtmp_pool.tile([128, QC], F32, tag="qt32", bufs=16, name=f"d{mt}_{q}")
                    nc.gpsimd.tensor_tensor(out=d[:], in0=c[:], in1=w[:], op=ALU.subtract)
                    s = qtmp_pool.tile([128, QC], F16, tag="qt16", bufs=12, name=f"s{mt}_{q}")
                    s_eng = nc.vector if q % 2 == 0 else nc.gpsimd
                    s_eng.tensor_tensor(out=s[:], in0=c[:], in1=d[:], op=ALU.subtract)

                    # low path: RNE to multiples of 0.5 on ACT (two affine copies)
                    u = qtmp_pool.tile([128, QC], F32, tag="qt32", bufs=16, name=f"u{mt}_{q}")
                    nc.scalar.activation(out=u[:], in_=w[:], func=ACT.Copy, bias=CR)
                    sL = qtmp_pool.tile([128, QC], F16, tag="qt16", bufs=12, name=f"sL{mt}_{q}")
                    nc.scalar.activation(out=sL[:], in_=u[:], func=ACT.Copy, bias=-CR)

                    # mask: low region iff |w| < sqrt(2)
                    aw = qtmp_pool.tile([128, QC], F16, tag="qt16", bufs=12, name=f"aw{mt}_{q}")
                    nc.scalar.activation(out=aw[:], in_=w[:], func=ACT.Abs)
                    mask = qtmp_pool.tile([128, QC], U8, tag="mask", bufs=6, name=f"mask{mt}_{q}")
                    nc.vector.tensor_scalar(out=mask[:], in0=aw[:], scalar1=THR,
                                            scalar2=None, op0=ALU.is_lt)
                    nc.vector.copy_predicated(out=s[:], mask=mask[:], data=sL[:])

                    xqc = xqc_pool.tile([128, QC], F16, tag="xqc", name=f"xqc{mt}_{q}")
                    nc.vector.tensor_tensor(
                        out=xqc.rearrange("p (b c) -> p b c", c=32),
                        in0=s.rearrange("p (b c) -> p b c", c=32),
                        in1=sc16.unsqueeze(2).broadcast_to([128, NB, 32]),
                        op=ALU.mult)

                    # transpose to K-major: PE identity transposes -> PSUM,
                    # then one strided DVE copy into xqT
                    pt = psumt_pool.tile([128, QC], F16, tag="tp", name=f"pt{mt}_{q}")
                    for j in range(TPC):
                        nc.tensor.transpose(pt[:, j * 128:(j + 1) * 128],
                                            xqc[:, j * 128:(j + 1) * 128], ident[:])
                    # contiguous destination in mt-major layout -> ACT copy
                    nc.scalar.copy(
                        out=xqT[:, mt * Kd + q * QC: mt * Kd + (q + 1) * QC],
                        in_=pt[:])

            # ---- Phase B: GEMM out[m, n] = sum_k xq[m, k] * WT[k, n] + bias ----
            def mm_block(psum_ap, mt, wts_nc, n0, nw):
                for k in range(KT):
                    nc.tensor.matmul(
                        out=psum_ap, lhsT=lhsT(k, mt), rhs=wts_nc[k][:],
                        start=(k == 0), stop=(k == KT - 1))

            def drain(psum_ap, mt, bnc, n0, nw, nci):
                ot = out_pool.tile([128, nw], F32, tag="ot", name=f"ot{nci}_{mt}")
                nc.vector.tensor_tensor(out=ot[:], in0=psum_ap, in1=bnc[:, :nw],
                                        op=ALU.add)
                nc.sync.dma_start(out=out[mt * 128:(mt + 1) * 128, n0:n0 + nw], in_=ot[:])

            def load_bias(nci, n0, nw):
                bnc = bias_pool.tile([128, nw], F32, tag="bnc", name=f"bnc{nci}")
                nc.sync.dma_start(
                    out=bnc[:],
                    in_=bias[n0:n0 + nw].unsqueeze(0).broadcast_to([128, nw]))
                return bnc

            def load_wts(nci, n0, nw):
                wts = []
                for k in range(KT):
                    wtt = wt_pool.tile([128, nw], F16, tag="wt", name=f"wt{nci}_{k}")
                    nc.sync.dma_start(out=wtt[:], in_=wt[k * 128:(k + 1) * 128, n0:n0 + nw])
                    wts.append(wtt)
                return wts

            # early section: first `early_nc` n-chunks, m-tile-major with
            # single-psum blocks, so PE work tracks quant production order
            early = []
            for nci in range(early_nc):
                n0, nw = nchunks[nci]
                early.append((nci, n0, nw, load_wts(nci, n0, nw), load_bias(nci, n0, nw)))
            for mt in range(MT):
                for nci, n0, nw, wts, bnc in early:
                    ps = psum_pool.tile([128, nw], F32, tag="ps", name=f"ps{nci}_{mt}")
                    mm_block(ps[:], mt, wts, n0, nw)
                    drain(ps[:], mt, bnc, n0, nw, nci)

            # steady state: waves of 4 m-tiles
            for nci in range(early_nc, len(nchunks)):
                n0, nw = nchunks[nci]
                wts = load_wts(nci, n0, nw)
                bnc = load_bias(nci, n0, nw)
                for g in range(0, MT, 4):
                    wave = list(range(g, min(g + 4, MT)))
                    psums = [
                        psum_pool.tile([128, nw], F32, tag="ps", name=f"ps{nci}_{mt}")
                        for mt in wave
                    ]
                    for k in range(KT):
                        for j, mt in enumerate(wave):
                            nc.tensor.matmul(
                                out=psums[j][:], lhsT=lhsT(k, mt), rhs=wts[k][:],
                                start=(k == 0), stop=(k == KT - 1))
                    for j, mt in enumerate(wave):
                        drain(psums[j][:], mt, bnc, n0, nw, nci)
    nc.compile()
    return nc


_CACHE = {}


def _get_program():
    if "nc" not in _CACHE:
        _CACHE["nc"] = build_program()
    return _CACHE["nc"]


def run(x, W, bias, trace=False):
    nc = _get_program()
    xf = np.ascontiguousarray(np.asarray(x, dtype=np.float32).reshape(M, K))
    WT16 = np.ascontiguousarray(np.asarray(W, dtype=np.float32).T.astype(np.float16))
    b32 = np.ascontiguousarray(np.asarray(bias, dtype=np.float32))
    in_maps = [
        {"x": xf[c * MS:(c + 1) * MS], "wt": WT16, "bias": b32}
        for c in range(N_CORES)
    ]
    res = run_bass_kernel_spmd(nc, in_maps, list(range(N_CORES)), trace=trace)
    outs = [res.results[c]["out"] for c in range(N_CORES)]
    full = np.concatenate(outs, axis=0).reshape(B, S, N)
    return full, res


def kernel(x, W, bias):
    out, _ = run(x, W, bias, trace=False)
    return out
